# revision 6
# baseline (speedup 1.0000x reference)
"""MinLSTM Trainium2 Bass kernel.

Problem (per batch element b, data-parallel over B=8 across 8 cores):
    gates = input @ W_hg                      # [T, 3H]
    f = sigmoid(gates[:, :H]); i = sigmoid(gates[:, H:2H]); ht = gates[:, 2H:]
    scan over t:  h_t = f_t * prev + i_t*ht_t,  prev = h0 if is_init_t else h_{t-1}
    out = h @ W_out ;  h_n = h[-1]

Device mapping (per core, one batch element):
  - matmul1 computes gates^T [3H, T] so the scan runs along the free dim
    with hidden features on partitions:  gates^T = W_hg^T @ input^T,
    lhsT = W_hg (natural layout), rhs = input^T (transposed on host).
  - ScalarE evacuates PSUM with fused sigmoid (f, i) / copy (ht) -> bf16.
  - VectorE: fm = f*m; a = f - fm; g = i*ht; b = fm*h0 + g (scalar_tensor_tensor);
    h = tensor_tensor_scan(a, b, initial=carry)  [state fp32 in hw].
  - matmul2: out[tc,OUT] = h_chunk @ W_out with lhsT = h^T (scan layout, fp32
    bitcast to float32r for full-rate PE), natural-layout contiguous store.
  - Recurrence resets are folded in as: a_t = (1-m_t)*f_t, b_t += m_t*f_t*h0.
"""

import os
from contextlib import ExitStack

import numpy as np
import ml_dtypes

import concourse.bass as bass
import concourse.bacc as bacc
import concourse.tile as tile
import concourse.mybir as mybir
from concourse.bass_utils import run_bass_kernel_spmd

B, T, IN, H, OUT = 8, 4096, 256, 512, 256
H3 = 3 * H
TC = 512               # time-chunk (free dim per matmul / PSUM bank)
NT = T // TC           # 8 chunks
NCC = H // 128         # 4 hidden-feature chunks of 128 partitions
NK = IN // 128         # 2 contraction chunks for matmul1
NM1 = H3 // 128        # 12 output-row chunks of gates^T
NS = TC // 128         # 4 t-subchunks (mm2 output rows)
N_CORES = 8

F32 = mybir.dt.float32
F32R = mybir.dt.float32r
BF16 = mybir.dt.bfloat16
PLANE_DT = BF16        # dtype of f/i/ht/a/b/m planes (DVE 2x mode)
MM_DT = F32R           # matmul operand view dtype (fp32 bits, full-rate PE)
Alu = mybir.AluOpType
Act = mybir.ActivationFunctionType


def build_program(reps=1):
    nc = bacc.Bacc(
        "TRN2", target_bir_lowering=False, debug=False, num_devices=N_CORES
    )
    xT = nc.dram_tensor("xT", [IN, T], MM_DT, kind="ExternalInput").ap()
    mrow = nc.dram_tensor("mrow", [1, T], PLANE_DT, kind="ExternalInput").ap()
    h0 = nc.dram_tensor("h0", [128, NCC], F32, kind="ExternalInput").ap()
    whg = nc.dram_tensor("whg", [IN, H3], MM_DT, kind="ExternalInput").ap()
    wout = nc.dram_tensor("wout", [H, OUT], MM_DT, kind="ExternalInput").ap()
    out = nc.dram_tensor("out", [T, OUT], F32, kind="ExternalOutput").ap()
    hn = nc.dram_tensor("hn", [128, NCC], MM_DT, kind="ExternalOutput").ap()

    with tile.TileContext(nc) as tc, ExitStack() as ctx:
        consts = ctx.enter_context(tc.tile_pool(name="consts", bufs=1))
        xpool = ctx.enter_context(tc.tile_pool(name="xpool", bufs=3))
        mpool = ctx.enter_context(tc.tile_pool(name="mpool", bufs=3))
        gates = ctx.enter_context(tc.tile_pool(name="gates", bufs=2))
        tmp = ctx.enter_context(tc.tile_pool(name="tmp", bufs=2))
        hpool = ctx.enter_context(tc.tile_pool(name="hpool", bufs=2 * NCC))
        opool = ctx.enter_context(tc.tile_pool(name="opool", bufs=2))
        gpsum = ctx.enter_context(tc.tile_pool(name="gpsum", bufs=4, space="PSUM"))
        opsum = ctx.enter_context(tc.tile_pool(name="opsum", bufs=2, space="PSUM"))

        # Preload weights / h0.
        w1 = consts.tile([128, NK, H3], MM_DT, tag="w1")
        for kk in range(NK):
            nc.sync.dma_start(out=w1[:, kk, :], in_=whg[kk * 128:(kk + 1) * 128, :])
        w2 = consts.tile([128, NCC, OUT], MM_DT, tag="w2")
        for c in range(NCC):
            nc.sync.dma_start(out=w2[:, c, :], in_=wout[c * 128:(c + 1) * 128, :])
        h0c = consts.tile([128, NCC], F32, tag="h0c")
        nc.sync.dma_start(out=h0c, in_=h0)

        h_prev = [None] * NCC
        for k in range(NT * reps):
            k = k % NT
            t0 = k * TC
            xt = xpool.tile([128, NK, TC], MM_DT, tag="xt")
            for kk in range(NK):
                nc.sync.dma_start(
                    out=xt[:, kk, :], in_=xT[kk * 128:(kk + 1) * 128, t0:t0 + TC]
                )
            mb = mpool.tile([128, TC], PLANE_DT, tag="mb")
            nc.gpsimd.dma_start(
                out=mb, in_=mrow[0:1, t0:t0 + TC].to_broadcast([128, TC])
            )

            f_sb = gates.tile([128, NCC, TC], PLANE_DT, tag="f")
            i_sb = gates.tile([128, NCC, TC], PLANE_DT, tag="i")
            ht_sb = gates.tile([128, NCC, TC], PLANE_DT, tag="ht")
            for m in range(NM1):
                pg = gpsum.tile([128, TC], F32, tag="pg")
                for kk in range(NK):
                    nc.tensor.matmul(
                        pg,
                        lhsT=w1[:, kk, m * 128:(m + 1) * 128],
                        rhs=xt[:, kk, :],
                        start=(kk == 0),
                        stop=(kk == NK - 1),
                    )
                if m < NCC:
                    nc.scalar.activation(out=f_sb[:, m, :], in_=pg, func=Act.Sigmoid)
                elif m < 2 * NCC:
                    nc.scalar.activation(
                        out=i_sb[:, m - NCC, :], in_=pg, func=Act.Sigmoid
                    )
                else:
                    nc.scalar.copy(out=ht_sb[:, m - 2 * NCC, :], in_=pg)

            h_cur = [None] * NCC
            for c in range(NCC):
                fm = tmp.tile([128, TC], PLANE_DT, tag="fm")
                nc.vector.tensor_mul(fm, f_sb[:, c, :], mb)
                a = tmp.tile([128, TC], PLANE_DT, tag="a")
                nc.vector.tensor_tensor(a, f_sb[:, c, :], fm, op=Alu.subtract)
                g = tmp.tile([128, TC], PLANE_DT, tag="g")
                nc.vector.tensor_mul(g, i_sb[:, c, :], ht_sb[:, c, :])
                b = tmp.tile([128, TC], PLANE_DT, tag="b")
                nc.vector.scalar_tensor_tensor(
                    b, fm, h0c[:, c:c + 1], g, op0=Alu.mult, op1=Alu.add
                )
                h = hpool.tile([128, TC], MM_DT, tag="h")
                init = h0c[:, c:c + 1] if k == 0 else h_prev[c][:, TC - 1:TC]
                nc.vector.tensor_tensor_scan(
                    h, a, b, init, op0=Alu.mult, op1=Alu.add
                )
                h_cur[c] = h

            osb = opool.tile([128, NS, OUT], F32, tag="osb")
            for s in range(NS):
                po = opsum.tile([128, OUT], F32, tag="po")
                for c in range(NCC):
                    nc.tensor.matmul(
                        po,
                        lhsT=h_cur[c][:, s * 128:(s + 1) * 128],
                        rhs=w2[:, c, :],
                        start=(c == 0),
                        stop=(c == NCC - 1),
                    )
                nc.scalar.copy(out=osb[:, s, :], in_=po)
                nc.sync.dma_start(
                    out=out[t0 + s * 128:t0 + (s + 1) * 128, :], in_=osb[:, s, :]
                )
            h_prev = h_cur

        for c in range(NCC):
            nc.sync.dma_start(out=hn[:, c:c + 1], in_=h_prev[c][:, TC - 1:TC])

    nc.compile()
    return nc


_CACHE = {}


def get_program():
    if "nc" not in _CACHE:
        _CACHE["nc"] = build_program(
            reps=int(os.environ.get("KERNEL_REPS", "1"))
        )
    return _CACHE["nc"]


def make_in_maps(input, is_init, h_0, W_hg, W_out):
    xT = np.ascontiguousarray(input.transpose(0, 2, 1)).astype(np.float32)
    mrow = np.ascontiguousarray(
        is_init.reshape(B, 1, T).astype(ml_dtypes.bfloat16)
    )
    h0r = np.ascontiguousarray(
        h_0.reshape(B, NCC, 128).transpose(0, 2, 1)
    ).astype(np.float32)
    whg = np.ascontiguousarray(W_hg).astype(np.float32)
    wout = np.ascontiguousarray(W_out).astype(np.float32)
    return [
        {"xT": xT[b], "mrow": mrow[b], "h0": h0r[b], "whg": whg, "wout": wout}
        for b in range(B)
    ]


def assemble_outputs(results):
    out = np.stack([results[b]["out"] for b in range(B)])
    hn = np.stack(
        [results[b]["hn"].transpose(1, 0).reshape(1, H) for b in range(B)]
    )
    return out.astype(np.float32), hn.astype(np.float32)


def kernel(input, is_init, h_0, W_hg, W_out):
    nc = get_program()
    in_maps = make_in_maps(input, is_init, h_0, W_hg, W_out)
    res = run_bass_kernel_spmd(
        nc,
        in_maps,
        core_ids=list(range(N_CORES)),
        trace=bool(int(os.environ.get("KERNEL_TRACE", "0"))),
    )
    _CACHE["last_results"] = res
    return assemble_outputs(res.results)
